# revision 47
# baseline (speedup 1.0000x reference)
"""Trainium2 Bass kernel for nn_AttentionBlock (linear attention + BatchNorm).

Math (per batch, c=256 channels, n=1024 pixels, 8 heads x 64 dims):
  qkv = w_qkv @ x                      [1536, n]
  q   = softmax(q, axis=d) * d^-0.5    (per head, over the 64 head-dims)
  k   = softmax(k, axis=n)             (per head-dim, over pixels)
  ctx = k @ (v/n)^T                    [d, e] per head
  out = ctx^T @ q                      [e, n] per head
  y   = BatchNorm(w_out @ out + b_out) (batch stats over (b, n) per channel)

Sharding: data-parallel over batch across 8 cores (4 batches each). BN batch
stats (mean, E[x^2] per channel; 2KB) are combined with one AllReduce; a
dependency-free dummy AllReduce at kernel start absorbs the ncfw rendezvous.
(An experimental SWDGE remote-DMA stats exchange is kept behind
BASS_ATTN_RDX=1 — it compiles but the runtime rejects it today.)
b_out is skipped: BatchNorm's mean subtraction cancels per-channel constants.

Scaling: out_sb values land ~1e-6 which is deep-subnormal fp16 (0.3-2%%
granularity). We fold a factor FOLD=1024 into the softmax-normalizer mask
(MASKVAL = n / SCALE / FOLD), making out_sb ~1e-3. The final projection is
then FOLD x larger, compensated exactly inside BatchNorm by using
eps' = eps * FOLD^2.  (CPU check: rel err 5.9e-3 -> 2.3e-3.)

PE-dense emission: ctx matmuls lag the kv matmuls by 2 chunks; Zq/out
matmuls interleave with the q projection; PSUM tiles round-robin across two
4-bank pools so write-after-read never stalls the PE. The out-stage packs
head pairs into one [128,128] block-diagonal lhsT (zeros off-diagonal), one
512-col matmul per half instead of two 64-partition ones.
"""

import os
import sys

import numpy as np

for _p in ("/opt/trn_rl_repo", "/root/.axon_site/_ro/trn_rl_repo"):
    if os.path.isdir(_p) and _p not in sys.path:
        sys.path.insert(0, _p)

import concourse.bacc as bacc
import concourse.tile as tile
from concourse import mybir
from concourse.bass_utils import run_bass_kernel_spmd

F32 = mybir.dt.float32
# bf16, not fp16: measured fp16 512-col matmuls stream at ~0.85ns/col on this
# silicon while bf16 is the documented full-rate path; CPU-checked rel err
# with bf16 operands everywhere is 6.9e-3 (tolerance 2e-2).
FP16 = mybir.dt.bfloat16
AF = mybir.ActivationFunctionType
ALU = mybir.AluOpType

N_CORES = 8
B = int(os.environ.get("BASS_ATTN_B", "4"))  # batches per core
C = 256          # channels
NPIX = 1024      # pixels (32*32)
H = 8            # heads
D = 64           # head dim
HID = H * D      # 512
NT = NPIX // 128  # 8 n-tiles
CT = C // 128     # 2 c-tiles
QT = HID // 128   # 4 q-tiles
SCALE = D ** -0.5
FOLD = 1024.0
BN_EPS = 1e-5 * FOLD * FOLD
MASKVAL = NPIX / SCALE / FOLD


def _emit(tc, x, wqkv, wout, gammab, betab, y):
    nc = tc.nc
    from contextlib import ExitStack
    ctx_stack = ExitStack()
    no_rdx = os.environ.get("BASS_ATTN_NO_RDX") == "1"
    with ctx_stack:
        const = ctx_stack.enter_context(tc.tile_pool(name="const", bufs=1))
        xin = ctx_stack.enter_context(tc.tile_pool(name="xin", bufs=4))
        kvsb = ctx_stack.enter_context(tc.tile_pool(name="kvsb", bufs=4))
        vxp = ctx_stack.enter_context(tc.tile_pool(name="vxp", bufs=NT))
        qpool = ctx_stack.enter_context(tc.tile_pool(name="qpool", bufs=5))
        rpool = ctx_stack.enter_context(tc.tile_pool(name="rpool", bufs=5))
        opool = ctx_stack.enter_context(tc.tile_pool(name="opool", bufs=6))
        fpool = ctx_stack.enter_context(tc.tile_pool(name="fpool", bufs=2 * B))
        small = ctx_stack.enter_context(tc.tile_pool(name="small", bufs=10))
        stats_p = ctx_stack.enter_context(tc.tile_pool(name="statsp", bufs=1))
        pbig = ctx_stack.enter_context(
            tc.tile_pool(name="pbig", bufs=4, space="PSUM"))
        pctx = ctx_stack.enter_context(
            tc.tile_pool(name="pctx", bufs=4, space="PSUM"))

        # round-robin PSUM allocator for late phase 2 (q2+/out/final):
        # alternates the two 4-bank pools (pctx first — its ctxu slots free
        # right after extraction) so a slot's reuse distance is 8 allocations.
        rr_state = [0]

        def psum_rr(name):
            pool = (pctx, pbig)[rr_state[0] & 1]
            rr_state[0] += 1
            return pool.tile([128, 512], F32, name=name, tag="big")

        # ---- semaphores for the remote stats exchange (experimental) ----
        use_rdx = os.environ.get("BASS_ATTN_RDX") == "1"
        if use_rdx:
            rsem = nc.alloc_semaphore("rdx_remote")
            lsem = nc.alloc_semaphore("rdx_local")
            # clear at kernel start: a peer's first send arrives ~80us into
            # its run, far after every core's start-of-run clear.
            rdx_clears = [nc.gpsimd.sem_clear(rsem),
                          nc.gpsimd.sem_clear(lsem)]
        dpool = ctx_stack.enter_context(
            tc.tile_pool(name="dram", bufs=1, space="DRAM"))
        if not no_rdx and not use_rdx:
            # dummy collective up front: the first AllReduce pays a large
            # ncfw rendezvous; running it early overlaps that with compute.
            # Its input is an uninitialized DRAM tile (value unused) so the
            # collective has no upstream deps — Tile's collective
            # serialization point stays at the very start of the kernel.
            wrm_i = dpool.tile([128, 1], F32, name="wrm_i")
            wrm_o = dpool.tile([128, 1], F32, name="wrm_o")
            nc.gpsimd.collective_compute(
                "AllReduce", ALU.add,
                replica_groups=[list(range(N_CORES))],
                ins=[wrm_i.opt()], outs=[wrm_o.opt()])

        # ---- constants ----
        wqkv_sb = []
        for kc in range(CT):
            w = const.tile([128, 3 * HID], FP16, name=f"wqkv{kc}")
            # kv columns first so the first batch's kv matmuls start early
            q_eng = (nc.sync, nc.scalar)[kc]
            q_eng.dma_start(out=w[:, HID:3 * HID],
                            in_=wqkv[128 * kc:128 * (kc + 1), HID:3 * HID])
            wqkv_sb.append(w)
        # batch 0's x right behind the kv weight columns, before everything
        # else, so the first matmul can start ~2us in
        xc0 = []
        for kc in range(CT):
            xt = xin.tile([128, NPIX], FP16, name="xc")
            (nc.scalar, nc.sync)[kc].dma_start(
                out=xt, in_=x[0, 128 * kc:128 * (kc + 1), :])
            xc0.append(xt)
        for kc in range(CT):
            (nc.sync, nc.scalar)[kc].dma_start(
                out=wqkv_sb[kc][:, 0:HID],
                in_=wqkv[128 * kc:128 * (kc + 1), 0:HID])
        wout_sb = []
        for k4 in range(HID // 128):
            w = const.tile([128, C], FP16, name=f"wout{k4}")
            nc.sync.dma_start(out=w, in_=wout[128 * k4:128 * (k4 + 1), :])
            wout_sb.append(w)
        gamma2 = const.tile([128, CT], F32, name="gamma2")
        beta2 = const.tile([128, CT], F32, name="beta2")
        for m in range(CT):
            nc.sync.dma_start(out=gamma2[:, m:m + 1],
                              in_=gammab[128 * m:128 * (m + 1), :])
            nc.sync.dma_start(out=beta2[:, m:m + 1],
                              in_=betab[128 * m:128 * (m + 1), :])
        bmask = const.tile([128, 128], FP16, name="bmask")
        nc.vector.memset(bmask, 0.0)
        nc.vector.memset(bmask[0:64, 0:64], MASKVAL)
        nc.vector.memset(bmask[64:128, 64:128], MASKVAL)
        eps_sb = const.tile([128, 1], F32, name="eps")
        nc.vector.memset(eps_sb, BN_EPS)
        # block-diagonal ctx tiles: zeros off-diagonal persist across batches
        cs_t = []
        for pr in range(QT):
            cs = const.tile([128, 128], FP16, name=f"cs{pr}")
            nc.vector.memset(cs, 0.0)
            cs_t.append(cs)
        # vx tiles: [128, pair, 130] = [v_h0 | ones | v_h1 | ones]; col 64
        # serves as the Zk ones-column for BOTH heads of the pair (rows 0:64
        # contract against h0, rows 64:128 against h1). One memset per slot;
        # v-casts only overwrite the v columns.
        vx_t = []
        for t in range(NT):
            vx = vxp.tile([128, QT, 2 * (D + 1)], FP16, name="vx")
            nc.vector.memset(vx, 1.0)
            vx_t.append(vx)

        stats_sb = [stats_p.tile([128, 2 * B, 6], F32, name=f"stats{m}",
                                 tag=f"stats{m}")
                    for m in range(CT)]
        final_sb = [[None] * CT for _ in range(B)]

        for b in range(B):
            if b == 0:
                xc = xc0
            else:
                xc = []
                for kc in range(CT):
                    xt = xin.tile([128, NPIX], FP16, name="xc")
                    x_eng = (nc.scalar, nc.sync)[kc]
                    x_eng.dma_start(out=xt,
                                    in_=x[b, 128 * kc:128 * (kc + 1), :])
                    xc.append(xt)

            # ---- KV projection + context accumulation (ctx lags 2 chunks) --
            # ctxu[pr] [128, 129]: rows 0:64 x cols 0:64 = ctx of head 2pr,
            # rows 64:128 x cols 65:129 = ctx of head 2pr+1; col 64 = Zk for
            # both (the shared ones-column). Off-diagonal blocks are garbage.
            ctxu = [pctx.tile([128, 2 * D + 1], F32, name="ctxu", tag="big")
                    for _ in range(QT)]
            expk = [None] * NT

            def emit_kv(t):
                kh = pbig.tile([128, 512], F32, name="kh", tag="big")
                vh = pbig.tile([128, 512], F32, name="vh", tag="big")
                for kc in range(CT):
                    nc.tensor.matmul(
                        kh,
                        lhsT=xc[kc][:, 128 * t:128 * (t + 1)],
                        rhs=wqkv_sb[kc][:, HID:HID + 512],
                        start=(kc == 0), stop=(kc == CT - 1))
                for kc in range(CT):
                    nc.tensor.matmul(
                        vh,
                        lhsT=xc[kc][:, 128 * t:128 * (t + 1)],
                        rhs=wqkv_sb[kc][:, HID + 512:HID + 1024],
                        start=(kc == 0), stop=(kc == CT - 1))
                ek = kvsb.tile([128, HID], FP16, name="expk")
                nc.scalar.activation(out=ek, in_=kh, func=AF.Exp)
                expk[t] = ek
                vx4 = vx_t[t].rearrange("p r (j e) -> p r j e", j=2)
                nc.vector.tensor_copy(
                    vx4[:, :, :, 0:D],
                    vh.rearrange("p (r j e) -> p r j e", r=QT, j=2))

            def emit_ctx(t):
                for pr in range(QT):
                    nc.tensor.matmul(
                        ctxu[pr],
                        lhsT=expk[t][:, 128 * pr:128 * (pr + 1)],
                        rhs=vx_t[t][:, pr, 0:2 * D + 1],
                        start=(t == 0), stop=(t == NT - 1))

            for t in range(NT):
                emit_kv(t)
                if 2 <= t <= 7:
                    emit_ctx(t - 2)

            # ---- Q projection (first two tiles fill the ctx-tail gap) ----
            eq_t, rb_t, qh_t = [None] * QT, [None] * QT, [None] * QT

            def emit_q(t, early=False):
                # early q tiles (t=0,1) must stay off pctx: its banks still
                # hold the open ctxu accumulation groups.
                eq = qpool.tile([128, NPIX], FP16, name="expq")
                qhs = []
                for nch in range(2):
                    qh = (pbig.tile([128, 512], F32, name="qh", tag="big")
                          if early else psum_rr("qh"))
                    for kc in range(CT):
                        nc.tensor.matmul(
                            qh,
                            lhsT=wqkv_sb[kc][:, 128 * t:128 * (t + 1)],
                            rhs=xc[kc][:, 512 * nch:512 * (nch + 1)],
                            start=(kc == 0), stop=(kc == CT - 1))
                    nc.scalar.activation(
                        out=eq[:, 512 * nch:512 * (nch + 1)], in_=qh,
                        func=AF.Exp)
                    qhs.append(qh)
                eq_t[t], qh_t[t] = eq, qhs

            def emit_zq(t):
                # Zqb overwrites qh in place (WAR after the exp read); the
                # reciprocal folds SCALE, 1/n and the FOLD rescale.
                rb = rpool.tile([128, NPIX], F32, name="recipb")
                for nch in range(2):
                    qh = qh_t[t][nch]
                    nc.tensor.matmul(
                        qh, lhsT=bmask,
                        rhs=eq_t[t][:, 512 * nch:512 * (nch + 1)],
                        start=True, stop=True)
                    nc.vector.reciprocal_approx_fast(
                        out=rb[:, 512 * nch:512 * (nch + 1)], in_=qh)
                rb_t[t] = rb

            def emit_extract():
                # ctx normalization into the block-diagonal cs tiles; col 64
                # of ctxu holds Zk for both heads of the pair.
                for pr in range(QT):
                    rz = small.tile([128, 1], F32, name="rz")
                    nc.vector.reciprocal_approx_fast(
                        out=rz, in_=ctxu[pr][:, D:D + 1])
                    nc.vector.tensor_scalar_mul(
                        cs_t[pr][0:64, 0:64],
                        in0=ctxu[pr][0:64, 0:D], scalar1=rz[0:64])
                    nc.vector.tensor_scalar_mul(
                        cs_t[pr][64:128, 64:128],
                        in0=ctxu[pr][64:128, D + 1:2 * D + 1],
                        scalar1=rz[64:128])

            def emit_out(t):
                os_ = opool.tile([128, NPIX], FP16, name="outsb")
                for nch in range(2):
                    oh = psum_rr("oh")
                    nc.tensor.matmul(
                        oh, lhsT=cs_t[t],
                        rhs=eq_t[t][:, 512 * nch:512 * (nch + 1)],
                        start=True, stop=True)
                    nc.vector.tensor_mul(
                        os_[:, 512 * nch:512 * (nch + 1)], oh,
                        rb_t[t][:, 512 * nch:512 * (nch + 1)])
                return os_

            emit_q(0, early=True)
            emit_ctx(6)
            emit_q(1, early=True)
            emit_ctx(7)
            emit_extract()
            rr_state[0] = 0
            emit_zq(0)
            emit_q(2)
            emit_zq(1)
            out_sb = [emit_out(0)]
            emit_q(3)
            emit_zq(2)
            out_sb.append(emit_out(1))
            emit_zq(3)
            out_sb.append(emit_out(2))
            out_sb.append(emit_out(3))

            # ---- final projection + bn stats ----
            for m in range(CT):
                fs = fpool.tile([128, NPIX], FP16, name="final")
                for nch in range(2):
                    fh = psum_rr("fh")
                    for k4 in range(HID // 128):
                        nc.tensor.matmul(
                            fh,
                            lhsT=wout_sb[k4][:, 128 * m:128 * (m + 1)],
                            rhs=out_sb[k4][:, 512 * nch:512 * (nch + 1)],
                            start=(k4 == 0), stop=(k4 == HID // 128 - 1))
                    fsh = fs[:, 512 * nch:512 * (nch + 1)]
                    nc.scalar.copy(fsh, fh)
                    # stats straight from PSUM: off the ACT-copy critical
                    # path (stats on unrounded f32 vs fp16 fs: ~5e-4, moot)
                    nc.vector.bn_stats(
                        out=stats_sb[m][:, 2 * b + nch, :], in_=fh)
                final_sb[b][m] = fs

        # ---- batch-norm stats: pack, remote exchange, reduce ----
        # switch the ACT table to the sqrt set while the PE tail still runs
        warm_rs = small.tile([1, 1], F32, name="warmrs")
        nc.scalar.activation(out=warm_rs, in_=eps_sb[0:1, :], func=AF.Sqrt)
        # pk4 layout: cols [0:2] = means (m0, m1); cols [2:4] = E[x^2]
        pk4 = small.tile([128, 2 * CT], F32, name="pk4")
        for m in range(CT):
            mv = small.tile([128, 2], F32, name="mv")
            nc.vector.bn_aggr(out=mv, in_=stats_sb[m])
            nc.vector.tensor_mul(pk4[:, CT + m:CT + m + 1],
                                 mv[:, 0:1], mv[:, 0:1])
            nc.vector.tensor_add(pk4[:, CT + m:CT + m + 1],
                                 pk4[:, CT + m:CT + m + 1], mv[:, 1:2])
            nc.vector.tensor_copy(pk4[:, m:m + 1], mv[:, 0:1])
        nc.vector.tensor_scalar_mul(pk4, in0=pk4, scalar1=1.0 / N_CORES)

        gst = small.tile([128, 2 * CT], F32, name="gst")
        if use_rdx:
            slots = stats_p.tile([128, N_CORES, 2 * CT], F32, name="slots",
                                 tag="slots")
            nc.vector.tensor_copy(slots[:, 0, :], pk4)  # self slot
            from concourse.instruction_name_ordered_set import (
                InstructionNameOrderedSet)
            clear_names = InstructionNameOrderedSet()
            for c in rdx_clears:
                clear_names.add(c.ins.name)
            for r in range(1, N_CORES):
                # slot r <- stats of peer (me XOR r); slot index == r keeps
                # cross-die dests on D2D-capable engine lanes.
                rdests = [(0, r) if kk == r else None for kk in range(N_CORES)]
                prep = nc.gpsimd.remote_dma_broadcast(
                    out_ap=slots[:, r, :], in_ap=pk4,
                    remote_sem=rsem, local_sem=lsem, rdests=rdests)
                # order-only edge: the run-start sem clears must precede the
                # sends (Tile may otherwise reorder within the engine queue)
                prep.ins.add_nosync_dependencies_from(clear_names)
            # signals_writable marks the remote-landing region as written by
            # the trigger, ordering the reduce's reads after it.
            trig = nc.gpsimd.trigger_dma(
                count=None, signals_writable=[slots[:, 1:N_CORES, :]])
            # arrival gate: emitted with value 0 (trivially true, so the
            # single-core scheduler sim does not falsely deadlock); patched
            # to 14 after scheduling (see _build).
            wgate = nc.gpsimd.wait_ge(rsem, 0)
            tn = InstructionNameOrderedSet()
            tn.add(trig.ins.name)
            wgate.ins.add_nosync_dependencies_from(tn)
            _PENDING_WAITS.append((wgate, rsem.num, 2 * (N_CORES - 1)))
            s8 = slots.rearrange("p r c -> p (r c)")
            h4 = stats_p.tile([128, 4 * CT * 2], F32, name="h4", tag="h4")
            a1 = nc.gpsimd.tensor_add(h4, s8[:, 0:4 * 2 * CT],
                                      s8[:, 4 * 2 * CT:8 * 2 * CT])
            wn = InstructionNameOrderedSet()
            wn.add(wgate.ins.name)
            a1.ins.add_nosync_dependencies_from(wn)
            nc.gpsimd.tensor_add(h4[:, 0:2 * 2 * CT], h4[:, 0:2 * 2 * CT],
                                 h4[:, 2 * 2 * CT:4 * 2 * CT])
            nc.gpsimd.tensor_add(gst, h4[:, 0:2 * CT],
                                 h4[:, 2 * CT:2 * 2 * CT])
        elif no_rdx:
            nc.vector.tensor_copy(gst, pk4)
        else:
            ccin = dpool.tile([128, 2 * CT], F32, name="ccin")
            ccout = dpool.tile([128, 2 * CT], F32, name="ccout")
            nc.sync.dma_start(out=ccin, in_=pk4)
            nc.gpsimd.collective_compute(
                "AllReduce", ALU.add,
                replica_groups=[list(range(N_CORES))],
                ins=[ccin.opt()], outs=[ccout.opt()])
            nc.sync.dma_start(out=gst, in_=ccout)

        # ---- normalize scalars (both m at once), normalize, store ----
        gmean2 = gst[:, 0:CT]
        gex22 = gst[:, CT:2 * CT]
        var2 = small.tile([128, CT], F32, name="var2")
        nc.vector.tensor_mul(var2, gmean2, gmean2)
        nc.vector.tensor_sub(var2, gex22, var2)
        std2 = small.tile([128, CT], F32, name="std2")
        nc.scalar.activation(out=std2, in_=var2, func=AF.Sqrt, bias=eps_sb)
        rstd2 = small.tile([128, CT], F32, name="rstd2")
        nc.vector.reciprocal_approx_fast(out=rstd2, in_=std2)
        rsg2 = small.tile([128, CT], F32, name="rsg2")
        nc.vector.tensor_mul(rsg2, rstd2, gamma2)
        sh2 = small.tile([128, CT], F32, name="sh2")
        nc.vector.tensor_mul(sh2, gmean2, rsg2)
        nc.vector.tensor_sub(sh2, beta2, sh2)
        for m in range(CT):
            rsg = rsg2[:, m:m + 1]
            sh = sh2[:, m:m + 1]
            for b in range(B):
                # gpsimd is deliberately idle in the tail: its tensor_scalar
                # is ~3x slower than DVE and any queued DMA lengthens the
                # end-of-kernel gpsimd drain. ACT takes 2 of 8 (1.2us each).
                fs = final_sb[b][m]
                if (2 * b + m) % 8 in (2, 5):
                    nc.scalar.activation(
                        out=fs, in_=fs, func=AF.Identity,
                        bias=sh, scale=rsg)
                else:
                    nc.vector.tensor_scalar(
                        out=fs, in0=fs, scalar1=rsg, scalar2=sh,
                        op0=ALU.mult, op1=ALU.add)
                dma_eng = (nc.sync, nc.scalar)[(2 * b + m) % 2]
                dma_eng.dma_start(
                    out=y[b, 128 * m:128 * (m + 1), :], in_=fs)


_CACHE = {}
# (instruction, sem, value) waits attached after Tile scheduling: the
# scheduler's single-core sim cannot see remote increments and would
# falsely report a deadlock if it simulated them.
_PENDING_WAITS = []


def _build():
    if "nc" in _CACHE:
        return _CACHE["nc"]
    nc = bacc.Bacc("TRN2", target_bir_lowering=False, debug=False,
                   enable_asserts=True, num_devices=N_CORES)
    x = nc.dram_tensor("x", [B, C, NPIX], FP16, kind="ExternalInput").ap()
    wqkv = nc.dram_tensor("wqkvT", [C, 3 * HID], FP16,
                          kind="ExternalInput").ap()
    wout = nc.dram_tensor("woutT", [HID, C], FP16, kind="ExternalInput").ap()
    gammab = nc.dram_tensor("gammab", [C, 1], F32, kind="ExternalInput").ap()
    betab = nc.dram_tensor("betab", [C, 1], F32, kind="ExternalInput").ap()
    y = nc.dram_tensor("y", [B, C, NPIX], FP16, kind="ExternalOutput").ap()
    _PENDING_WAITS.clear()
    with tile.TileContext(nc) as tc:
        _emit(tc, x, wqkv, wout, gammab, betab, y)
    for inst, sem_id, val in _PENDING_WAITS:
        inst.ins.sync_info.on_wait[0].wait_value = val
    nc.compile()
    for _inst, sem_id, val in _PENDING_WAITS:
        # the patched arrival gate must survive into the final program (the
        # event-semaphore may be renamed/fused during compile): some
        # instruction must still wait on this sem with the patched value.
        assert any(
            w.id == sem_id and w.wait_value == val
            for bb in nc.m.functions[0].blocks for i in bb.instructions
            if i.sync_info is not None for w in i.sync_info.on_wait), \
            "rdx arrival gate was optimized away"
    _PENDING_WAITS.clear()
    _CACHE["nc"] = nc
    return nc


def kernel(x, w_qkv, w_out, b_out, gamma, beta, _trace=False):
    x = np.asarray(x, dtype=np.float32)
    from ml_dtypes import bfloat16 as np_bf16
    wqkvT = np.ascontiguousarray(np.asarray(w_qkv, np_bf16).T)   # [256, 1536]
    woutT = np.ascontiguousarray(np.asarray(w_out, np_bf16).T)   # [512, 256]
    gammab = np.ascontiguousarray(np.asarray(gamma, np.float32).reshape(C, 1))
    betab = np.ascontiguousarray(np.asarray(beta, np.float32).reshape(C, 1))
    # b_out is intentionally unused: BatchNorm's mean subtraction cancels any
    # per-channel constant added before it, exactly.

    btot, c, hh, ww = x.shape
    assert (btot, c, hh * ww) == (B * N_CORES, C, NPIX)
    xf = x.reshape(btot, C, NPIX)

    nc = _build()
    in_maps = []
    for core in range(N_CORES):
        in_maps.append({
            "x": np.ascontiguousarray(xf[B * core:B * (core + 1)]).astype(np_bf16),
            "wqkvT": wqkvT,
            "woutT": woutT,
            "gammab": gammab,
            "betab": betab,
        })
    res = run_bass_kernel_spmd(nc, in_maps, core_ids=list(range(N_CORES)),
                               trace=_trace)
    y = np.concatenate([res.results[core]["y"] for core in range(N_CORES)],
                       axis=0)
    out = y.astype(np.float32).reshape(btot, C, hh, ww)
    if _trace:
        kernel.last_result = res
    return out
